# revision 5
# baseline (speedup 1.0000x reference)
"""Trainium2 Bass kernel for nn_ChemGeomFeatEncoder.

Strategy
--------
Host side: sort neighbor rows by vertex id, shard vertices (and their
neighbor rows) across the 8 cores in 256-vertex blocks, pad each block's
rows to a uniform R (multiple of 128).  Weights get eval-BatchNorm folded
into them, so every layer is matmul + bias + activation.

Device side (per core, one SPMD program):
  phase 1 (chem rows, feature-major [feat, rows] tiles):
    h1 = silu(W1x+b) via Exp(-p) [ACT] + (p)/(1+e^-p) [DVE stt divide]
    h2a/h2b = W2a/W2b @ h1;  h_chem = sigmoid(a)*softplus(b)
       = Ln(e^b + 1) / (1 + e^-a)   (everything in the exp/ln ACT table set)
    PE-transpose h_chem -> row-major, DMA rows out (h_chem output), and
    segment-sum via one-hot matmul into per-block PSUM accumulators.
    One-hot built on GPSIMD from iota-vs-vid compare.
  phase 2 (vertices): geom MLP + feat MLP on [feat, verts] tiles, seg
    stays resident in SBUF.  All matmuls f32r (1 cyc/row at FD>=256).

Outputs: h_geom [V,128] f32, h_chem [N,128] f32 (tuple, like reference).
"""

import numpy as np

import concourse.bass as bass
import concourse.mybir as mybir
import concourse.tile as tile
from concourse import bacc
from concourse.bass_utils import run_bass_kernel_spmd
from concourse.masks import make_identity

# problem constants (hardcoded per spec)
H = 128
CF = 128
GF = 64
N_ROWS = 500_000
V = 100_000
EPS = 1e-5

NCORES = 8
BLK = 256                    # vertices per segment-matmul block
NBLK = 49                    # blocks per core (8*49*256 = 100352 >= V)
VSH = NBLK * BLK             # vertices per core shard (12544)

F32 = mybir.dt.float32
F32R = mybir.dt.float32r

LAST_RESULTS = None          # test harness reads profiling info from here
TRACE = False

AF = mybir.ActivationFunctionType
ALU = mybir.AluOpType


def _fold_bn(W, b, g, be, m, v):
    rs = (g / np.sqrt(v + EPS)).astype(np.float32)
    return (W * rs[None, :]).astype(np.float32), ((b - m) * rs + be).astype(np.float32)


def _host_prepare(nbr, chem, geom):
    """Sort rows by vertex, shard into per-core block-padded layouts."""
    nbr = np.ascontiguousarray(nbr).astype(np.int64)
    order = np.argsort(nbr, kind='stable')
    svids = nbr[order]

    nblocks_total = NCORES * NBLK
    blk_of_row = svids // BLK
    counts = np.bincount(blk_of_row, minlength=nblocks_total).astype(np.int64)
    R = int(np.ceil(max(counts.max(), 128) / 128) * 128)

    block_start = np.zeros(nblocks_total, np.int64)
    block_start[1:] = np.cumsum(counts)[:-1]
    row_slot = np.arange(len(svids)) - block_start[blk_of_row]
    padded_pos = blk_of_row * R + row_slot
    RT = R * NBLK

    perm = np.full(nblocks_total * R, -1, np.int64)
    perm[padded_pos] = order

    cores = []
    for k in range(NCORES):
        p = perm[k * RT:(k + 1) * RT]
        real = p >= 0
        xk = np.zeros((RT, CF), np.float32)
        xk[real] = chem[p[real]]
        vl = np.full(RT, -1.0, np.float32)
        vl[real] = (nbr[p[real]] % BLK).astype(np.float32)
        gk = np.zeros((VSH, GF), np.float32)
        nreal = max(0, min(VSH, V - k * VSH))
        gk[:nreal] = geom[k * VSH:k * VSH + nreal]
        nch = RT // 128
        cores.append(dict(
            xT=np.ascontiguousarray(xk.T),                        # [CF, RT]
            vch=np.ascontiguousarray(vl.reshape(nch, 128).T),     # [128, NCH]
            gT=np.ascontiguousarray(gk.T),                        # [GF, VSH]
            perm=p,
        ))
    return cores, R


def _build_program(R, W):
    """Emit the Bass/Tile program for one core (all cores run it SPMD)."""
    RT = R * NBLK
    NCHB = R // 128                 # chunks per block
    NCH = NCHB * NBLK

    nc = bacc.Bacc("TRN2", target_bir_lowering=False, debug=False)

    xt = nc.dram_tensor("xt", [CF, RT], F32, kind="ExternalInput")
    vch = nc.dram_tensor("vch", [128, NCH], F32, kind="ExternalInput")
    gt = nc.dram_tensor("gt", [GF, VSH], F32, kind="ExternalInput")
    hc = nc.dram_tensor("hc", [RT, H], F32, kind="ExternalOutput")
    hg = nc.dram_tensor("hg", [H, VSH], F32, kind="ExternalOutput")

    # constants embedded in the NEFF
    ni = np.tile(np.arange(BLK, dtype=np.float32)[None, :], (128, 1))
    biases = np.stack([
        W['cb1'], -W['cb1'], -W['cb2a'], W['cb2b'],
        W['gb1'], -W['gb1'], W['gb2'],
        W['fb1'], -W['fb1'], W['fb2'],
    ], axis=1).astype(np.float32)                                 # [128, 10]
    inl = {}
    for name, arr in [('w_c1', W['cW1']), ('w_c2a', W['cW2a']), ('w_c2b', W['cW2b']),
                      ('w_g1', W['gW1']), ('w_g2', W['gW2']),
                      ('w_f1a', W['fW1a']), ('w_f1b', W['fW1b']), ('w_f2', W['fW2']),
                      ('ni', ni), ('bia', biases)]:
        inl[name] = nc.inline_tensor(np.ascontiguousarray(arr), name)

    with tile.TileContext(nc) as tc:
        with (
            tc.tile_pool(name="const", bufs=1) as cpool,
            tc.tile_pool(name="xin", bufs=3) as xpool,
            tc.tile_pool(name="work", bufs=2) as wpool,
            tc.tile_pool(name="oh", bufs=4) as ohpool,
            tc.tile_pool(name="segsb", bufs=4) as segsbpool,
            tc.tile_pool(name="p2", bufs=2) as p2pool,
            tc.tile_pool(name="ps_l1", bufs=2, space="PSUM") as ps_l1,
            tc.tile_pool(name="ps_a", bufs=1, space="PSUM") as ps_a,
            tc.tile_pool(name="ps_b", bufs=1, space="PSUM") as ps_b,
            tc.tile_pool(name="ps_t", bufs=2, space="PSUM") as ps_t,
            tc.tile_pool(name="ps_seg", bufs=2, space="PSUM") as ps_seg,
        ):
            # --- load constants ---
            def cload(name, shape, dt=F32):
                t = cpool.tile(shape, dt, tag=name)
                src_ap = inl[name].ap()
                if dt == F32R:
                    src_ap = src_ap.bitcast(F32R)
                nc.sync.dma_start(out=t[:], in_=src_ap)
                return t
            w_c1 = cload('w_c1', [CF, H], F32R)
            w_c2a = cload('w_c2a', [H, H], F32R)
            w_c2b = cload('w_c2b', [H, H], F32R)
            w_g1 = cload('w_g1', [GF, H], F32R)
            w_g2 = cload('w_g2', [H, H], F32R)
            w_f1a = cload('w_f1a', [H, H], F32R)
            w_f1b = cload('w_f1b', [H, H], F32R)
            w_f2 = cload('w_f2', [H, H], F32R)
            ni_sb = cload('ni', [128, BLK])
            bia = cload('bia', [128, 10])
            vch_sb = cpool.tile([128, NCH], F32, tag="vch")
            nc.sync.dma_start(out=vch_sb[:], in_=vch.ap())
            ident = cpool.tile([128, 128], F32, tag="ident")
            make_identity(nc, ident[:])

            cb1, ncb1, ncb2a, cb2b = (bia[:, i:i + 1] for i in range(4))
            gb1, ngb1, gb2 = (bia[:, i:i + 1] for i in range(4, 7))
            fb1, nfb1, fb2 = (bia[:, i:i + 1] for i in range(7, 10))

            # group splits of a block's NCHB chunks into <=4-chunk matmul groups
            splits = []
            off = 0
            while off < NCHB:
                gw = min(4, NCHB - off)
                splits.append((off, gw))
                off += gw

            seg_tiles = [None] * NBLK

            def phase2_tile(t):
                vbase = t * 512
                vw = min(512, VSH - vbase)
                nblk_here = vw // BLK
                gx = p2pool.tile([GF, vw], F32R, tag="gx")
                nc.sync.dma_start(out=gx[:], in_=gt[:, vbase:vbase + vw].bitcast(F32R))
                gl1 = ps_l1.tile([128, vw], F32, tag="l1")
                nc.tensor.matmul(gl1[:], w_g1[:], gx[:], start=True, stop=True)
                gep = p2pool.tile([128, vw], F32, tag="gep")
                nc.scalar.activation(gep[:], gl1[:], AF.Exp, bias=ngb1, scale=-1.0)
                gden = p2pool.tile([128, vw], F32, tag="gden")
                nc.gpsimd.tensor_scalar(gden[:], gep[:], 1.0, None, ALU.add)
                grec = p2pool.tile([128, vw], F32, tag="grec")
                nc.vector.reciprocal(grec[:], gden[:])
                g1 = p2pool.tile([128, vw], F32R, tag="g1")
                nc.vector.scalar_tensor_tensor(g1[:], gl1[:], gb1, grec[:],
                                               ALU.add, ALU.mult)
                gl2 = ps_a.tile([128, vw], F32, tag="a")
                nc.tensor.matmul(gl2[:], w_g2[:], g1[:], start=True, stop=True)
                g2 = p2pool.tile([128, vw], F32R, tag="g2")
                nc.scalar.activation(g2[:], gl2[:], AF.Identity, bias=gb2, scale=1.0)

                fl1 = ps_b.tile([128, vw], F32, tag="b")
                nc.tensor.matmul(fl1[:], w_f1b[:], g2[:],
                                 start=True, stop=False, skip_group_check=True)
                for j in range(nblk_here):
                    nc.tensor.matmul(fl1[:, j * BLK:(j + 1) * BLK],
                                     w_f1a[:], seg_tiles[2 * t + j][:],
                                     start=False, stop=True, skip_group_check=True)
                fep = p2pool.tile([128, vw], F32, tag="fep")
                nc.scalar.activation(fep[:], fl1[:], AF.Exp, bias=nfb1, scale=-1.0)
                fden = p2pool.tile([128, vw], F32, tag="fden")
                nc.gpsimd.tensor_scalar(fden[:], fep[:], 1.0, None, ALU.add)
                frec = p2pool.tile([128, vw], F32, tag="frec")
                nc.vector.reciprocal(frec[:], fden[:])
                f1 = p2pool.tile([128, vw], F32R, tag="f1")
                nc.vector.scalar_tensor_tensor(f1[:], fl1[:], fb1, frec[:],
                                               ALU.add, ALU.mult)
                fl2 = ps_t.tile([128, vw], F32, tag="t")
                nc.tensor.matmul(fl2[:], w_f2[:], f1[:], start=True, stop=True)
                out = p2pool.tile([128, vw], F32, tag="out")
                nc.scalar.activation(out[:], fl2[:], AF.Identity, bias=fb2, scale=1.0)
                nc.sync.dma_start(out=hg[:, vbase:vbase + vw], in_=out[:])

            for b in range(NBLK):
                segps = ps_seg.tile([128, BLK], F32, tag="seg")
                for (goff, gw) in splits:
                    w = gw * 128
                    c0 = b * NCHB + goff        # global chunk index of group start
                    xtile = xpool.tile([CF, w], F32R, tag="x")
                    nc.sync.dma_start(out=xtile[:],
                                      in_=xt[:, c0 * 128:c0 * 128 + w].bitcast(F32R))
                    l1 = ps_l1.tile([128, w], F32, tag="l1")
                    nc.tensor.matmul(l1[:], w_c1[:], xtile[:], start=True, stop=True)
                    ep = wpool.tile([128, w], F32, tag="ep")
                    nc.scalar.activation(ep[:], l1[:], AF.Exp, bias=ncb1, scale=-1.0)
                    den1 = wpool.tile([128, w], F32, tag="den1")
                    nc.gpsimd.tensor_scalar(den1[:], ep[:], 1.0, None, ALU.add)
                    rec1 = wpool.tile([128, w], F32, tag="rec1")
                    nc.vector.reciprocal(rec1[:], den1[:])
                    h1 = wpool.tile([128, w], F32R, tag="h1")
                    nc.vector.scalar_tensor_tensor(h1[:], l1[:], cb1, rec1[:],
                                                   ALU.add, ALU.mult)
                    l2a = ps_a.tile([128, w], F32, tag="a")
                    nc.tensor.matmul(l2a[:], w_c2a[:], h1[:], start=True, stop=True)
                    l2b = ps_b.tile([128, w], F32, tag="b")
                    nc.tensor.matmul(l2b[:], w_c2b[:], h1[:], start=True, stop=True)
                    ea = wpool.tile([128, w], F32, tag="ea")
                    nc.scalar.activation(ea[:], l2a[:], AF.Exp, bias=ncb2a, scale=-1.0)
                    den2 = wpool.tile([128, w], F32, tag="den2")
                    nc.gpsimd.tensor_scalar(den2[:], ea[:], 1.0, None, ALU.add)
                    eb = wpool.tile([128, w], F32, tag="eb")
                    nc.scalar.activation(eb[:], l2b[:], AF.Exp, bias=cb2b, scale=1.0)
                    sp = wpool.tile([128, w], F32, tag="sp")
                    nc.scalar.activation(sp[:], eb[:], AF.Ln, bias=1.0, scale=1.0)
                    rec2 = wpool.tile([128, w], F32, tag="rec2")
                    nc.vector.reciprocal(rec2[:], den2[:])
                    hch = wpool.tile([128, w], F32, tag="hch")
                    nc.vector.tensor_tensor(out=hch[:], in0=sp[:], in1=rec2[:],
                                            op=ALU.mult)
                    tp = ps_t.tile([128, w], F32, tag="t")
                    for i in range(gw):
                        s = slice(i * 128, (i + 1) * 128)
                        nc.tensor.transpose(tp[:, s], hch[:, s], ident[:])
                    hrm = wpool.tile([128, w], F32R, tag="hrm")
                    nc.any.tensor_copy(out=hrm[:], in_=tp[:])
                    for i in range(gw):
                        s = slice(i * 128, (i + 1) * 128)
                        r0 = (c0 + i) * 128
                        nc.sync.dma_start(out=hc[r0:r0 + 128, :].bitcast(F32R),
                                          in_=hrm[:, s])
                    for i in range(gw):
                        c = c0 + i
                        oh = ohpool.tile([128, BLK], F32R, tag="oh")
                        nc.gpsimd.tensor_scalar(oh[:], ni_sb[:], vch_sb[:, c:c + 1],
                                                0.0, ALU.subtract, ALU.is_equal)
                        ci = goff + i           # chunk index within block
                        nc.tensor.matmul(segps[:], hrm[:, i * 128:(i + 1) * 128], oh[:],
                                         start=(ci == 0), stop=(ci == NCHB - 1))
                seg_sb = segsbpool.tile([128, BLK], F32R, tag="segsb")
                nc.any.tensor_copy(out=seg_sb[:], in_=segps[:])
                seg_tiles[b] = seg_sb
                if b % 2 == 1:
                    phase2_tile(b // 2)
            if NBLK % 2 == 1:
                phase2_tile(NBLK // 2)  # last 256-wide tile (block NBLK-1)

    nc.compile()
    return nc


def kernel(**inputs):
    global LAST_RESULTS
    i = {k: np.asarray(v) for k, v in inputs.items()}
    chem = np.ascontiguousarray(i['chem_feats'], dtype=np.float32)
    geom = np.ascontiguousarray(i['geom_feats'], dtype=np.float32)
    cores, R = _host_prepare(i['nbr_vids'], chem, geom)

    W = {}
    W['cW1'], W['cb1'] = _fold_bn(i['cW1'], i['cb1'], i['cg1'], i['cbe1'], i['cm1'], i['cv1'])
    cW2, cb2 = _fold_bn(i['cW2'], i['cb2'], i['cg2'], i['cbe2'], i['cm2'], i['cv2'])
    W['cW2a'], W['cb2a'] = np.ascontiguousarray(cW2[:, :H]), cb2[:H]
    W['cW2b'], W['cb2b'] = np.ascontiguousarray(cW2[:, H:]), cb2[H:]
    W['gW1'], W['gb1'] = _fold_bn(i['gW1'], i['gb1'], i['gg1'], i['gbe1'], i['gm1'], i['gv1'])
    W['gW2'], W['gb2'] = _fold_bn(i['gW2'], i['gb2'], i['gg2'], i['gbe2'], i['gm2'], i['gv2'])
    fW1, W['fb1'] = _fold_bn(i['fW1'], i['fb1'], i['fg1'], i['fbe1'], i['fm1'], i['fv1'])
    W['fW1a'], W['fW1b'] = np.ascontiguousarray(fW1[:H]), np.ascontiguousarray(fW1[H:])
    W['fW2'], W['fb2'] = _fold_bn(i['fW2'], i['fb2'], i['fg2'], i['fbe2'], i['fm2'], i['fv2'])

    nc = _build_program(R, W)

    in_maps = [{'xt': c['xT'], 'vch': c['vch'], 'gt': c['gT']} for c in cores]
    res = run_bass_kernel_spmd(nc, in_maps, core_ids=list(range(NCORES)), trace=TRACE)
    LAST_RESULTS = res

    h_chem = np.empty((N_ROWS, H), np.float32)
    h_geom = np.empty((V, H), np.float32)
    for k in range(NCORES):
        p = cores[k]['perm']
        real = p >= 0
        h_chem[p[real]] = res.results[k]['hc'][real]
        nreal = max(0, min(VSH, V - k * VSH))
        if nreal > 0:
            h_geom[k * VSH:k * VSH + nreal] = res.results[k]['hg'].T[:nreal]
    return h_geom, h_chem


# revision 19
# speedup vs baseline: 131.9251x; 131.9251x over previous
"""Trainium2 Bass kernel for nn_ChemGeomFeatEncoder.

Strategy
--------
Host side: sort neighbor rows by vertex id, shard vertices (and their
neighbor rows) across the 8 cores in 256-vertex blocks, pad each block's
rows to a uniform R (multiple of 128).  Weights get eval-BatchNorm folded
into them, so every layer is matmul + bias + activation.

Device side (per core, one SPMD program):
  phase 1 (chem rows, feature-major [feat, rows] tiles):
    h1 = silu(W1x+b) = (p) * 1/(1+e^-p): Exp on ACT, +1 on GPSIMD,
    reciprocal + fused (psum+bias)*rec on DVE (HW has no divide ALU op).
    h2a/h2b = W2a/W2b @ h1;  h_chem = sigmoid(a)*softplus(b)
       = Ln(e^b + 1) * recip(1 + e^-a)  (single exp/ln ACT table set,
       pinned so zero table switches; block-wide ops amortize fixed costs)
    PE-transpose h_chem -> row-major, DMA rows out (h_chem output), and
    segment-sum via one-hot matmul into per-block PSUM accumulators.
    One-hot built on GPSIMD from iota-vs-vid compare.
  phase 2 (vertices): geom MLP + feat MLP on [feat, verts] tiles, seg
    stays resident in SBUF.  All matmuls f32r (1 cyc/row at FD>=256).

Outputs: h_geom [V,128] f32, h_chem [N,128] f32 (tuple, like reference).
"""

import numpy as np

import concourse.bass as bass
import concourse.mybir as mybir
import concourse.tile as tile
from concourse import bacc
from concourse.tile import add_dep_helper
from concourse.bass_utils import run_bass_kernel_spmd
from concourse.masks import make_identity

# Pin every ACT function to the one table set that contains all functions this
# kernel uses (exp, ln, identity, copy).  The default chooser maps each
# function to the first set containing it, which alternates between
# exp_and_others and natural_log across our Exp/Ln sequence and inserts ~300
# table loads (~2.7us each).  Emptying all other sets (order preserved, so
# act_func_set_id indices stay aligned with act_info.json) forces a single
# load at kernel start.
_orig_get_activation_tables = bacc.get_activation_tables


def _pinned_activation_tables(arch):
    t = _orig_get_activation_tables(arch)
    out = {}
    for name, fns in t.items():
        if name == 'natural_log_exp_and_others':
            out[name] = fns                       # exp, ln, identity, copy, ...
        elif name == 'sigmoid_and_others':
            out[name] = {f for f in fns if f == mybir.ActivationFunctionType.Sigmoid}
        else:
            out[name] = set()
    return out


bacc.get_activation_tables = _pinned_activation_tables

# problem constants (hardcoded per spec)
H = 128
CF = 128
GF = 64
N_ROWS = 500_000
V = 100_000
EPS = 1e-5

NCORES = 8
BLK = 256                    # vertices per segment-matmul block
NBLK = 49                    # blocks per core (8*49*256 = 100352 >= V)
VSH = NBLK * BLK             # vertices per core shard (12544)

F32 = mybir.dt.float32
F32R = mybir.dt.float32r

LAST_RESULTS = None          # test harness reads profiling info from here
LAST_NC = None               # compiled Bass program (for offline cost analysis)
TRACE = False
# scheduling knobs (TimelineSim-tuned)
BUFS = dict(xin=3, work=2, oh=4, segsb=4, p2=2,
            ps_l1=4, ps_a=1, ps_b=1, ps_t=1, ps_seg=1)

AF = mybir.ActivationFunctionType
ALU = mybir.AluOpType


def _fold_bn(W, b, g, be, m, v):
    rs = (g / np.sqrt(v + EPS)).astype(np.float32)
    return (W * rs[None, :]).astype(np.float32), ((b - m) * rs + be).astype(np.float32)


def _host_prepare(nbr, chem, geom):
    """Sort rows by vertex, shard into per-core block-padded layouts."""
    nbr = np.ascontiguousarray(nbr).astype(np.int64)
    order = np.argsort(nbr, kind='stable')
    svids = nbr[order]

    nblocks_total = NCORES * NBLK
    blk_of_row = svids // BLK
    counts = np.bincount(blk_of_row, minlength=nblocks_total).astype(np.int64)
    R = int(np.ceil(max(counts.max(), 128) / 128) * 128)

    block_start = np.zeros(nblocks_total, np.int64)
    block_start[1:] = np.cumsum(counts)[:-1]
    row_slot = np.arange(len(svids)) - block_start[blk_of_row]
    padded_pos = blk_of_row * R + row_slot
    RT = R * NBLK

    perm = np.full(nblocks_total * R, -1, np.int64)
    perm[padded_pos] = order

    cores = []
    for k in range(NCORES):
        p = perm[k * RT:(k + 1) * RT]
        real = p >= 0
        xk = np.zeros((RT, CF), np.float32)
        xk[real] = chem[p[real]]
        vl = np.full(RT, -1.0, np.float32)
        vl[real] = (nbr[p[real]] % BLK).astype(np.float32)
        gk = np.zeros((VSH, GF), np.float32)
        nreal = max(0, min(VSH, V - k * VSH))
        gk[:nreal] = geom[k * VSH:k * VSH + nreal]
        nch = RT // 128
        cores.append(dict(
            xT=np.ascontiguousarray(xk.T),                        # [CF, RT]
            vch=np.ascontiguousarray(vl.reshape(nch, 128).T),     # [128, NCH]
            gT=np.ascontiguousarray(gk.T),                        # [GF, VSH]
            perm=p,
        ))
    return cores, R


def _build_program(R, W):
    """Emit the Bass/Tile program for one core (all cores run it SPMD)."""
    RT = R * NBLK
    NCHB = R // 128                 # chunks per block
    NCH = NCHB * NBLK

    nc = bacc.Bacc("TRN2", target_bir_lowering=False, debug=False)

    xt = nc.dram_tensor("xt", [CF, RT], F32, kind="ExternalInput")
    vch = nc.dram_tensor("vch", [128, NCH], F32, kind="ExternalInput")
    gt = nc.dram_tensor("gt", [GF, VSH], F32, kind="ExternalInput")
    hc = nc.dram_tensor("hc", [RT, H], F32, kind="ExternalOutput")
    hg = nc.dram_tensor("hg", [H, VSH], F32, kind="ExternalOutput")

    # constants embedded in the NEFF
    ni = np.tile(np.arange(BLK, dtype=np.float32)[None, :], (128, 1))
    biases = np.stack([
        W['cb1'], -W['cb1'], -W['cb2a'], W['cb2b'],
        W['gb1'], -W['gb1'], W['gb2'],
        W['fb1'], -W['fb1'], W['fb2'],
    ], axis=1).astype(np.float32)                                 # [128, 10]
    inl = {}
    for name, arr in [('w_c1', W['cW1']), ('w_c2a', W['cW2a']), ('w_c2b', W['cW2b']),
                      ('w_g1', W['gW1']), ('w_g2', W['gW2']),
                      ('w_f1a', W['fW1a']), ('w_f1b', W['fW1b']), ('w_f2', W['fW2']),
                      ('ni', ni), ('bia', biases)]:
        inl[name] = nc.inline_tensor(np.ascontiguousarray(arr), name)

    with tile.TileContext(nc) as tc:
        with (
            tc.tile_pool(name="const", bufs=1) as cpool,
            tc.tile_pool(name="xin", bufs=BUFS['xin']) as xpool,
            tc.tile_pool(name="work", bufs=BUFS['work']) as wpool,
            tc.tile_pool(name="oh", bufs=BUFS['oh']) as ohpool,
            tc.tile_pool(name="segsb", bufs=BUFS['segsb']) as segsbpool,
            tc.tile_pool(name="p2", bufs=BUFS['p2']) as p2pool,
            tc.tile_pool(name="ps_l1", bufs=BUFS['ps_l1'], space="PSUM") as ps_l1,
            tc.tile_pool(name="ps_a", bufs=BUFS['ps_a'], space="PSUM") as ps_a,
            tc.tile_pool(name="ps_b", bufs=BUFS['ps_b'], space="PSUM") as ps_b,
            tc.tile_pool(name="ps_t", bufs=BUFS['ps_t'], space="PSUM") as ps_t,
            tc.tile_pool(name="ps_seg", bufs=BUFS['ps_seg'], space="PSUM") as ps_seg,
        ):
            # --- load constants ---
            def cload(name, shape, dt=F32):
                t = cpool.tile(shape, dt, tag=name)
                src_ap = inl[name].ap()
                if dt == F32R:
                    src_ap = src_ap.bitcast(F32R)
                nc.sync.dma_start(out=t[:], in_=src_ap)
                return t
            w_c1 = cload('w_c1', [CF, H], F32R)
            w_c2a = cload('w_c2a', [H, H], F32R)
            w_c2b = cload('w_c2b', [H, H], F32R)
            w_g1 = cload('w_g1', [GF, H], F32R)
            w_g2 = cload('w_g2', [H, H], F32R)
            w_f1a = cload('w_f1a', [H, H], F32R)
            w_f1b = cload('w_f1b', [H, H], F32R)
            w_f2 = cload('w_f2', [H, H], F32R)
            ni_sb = cload('ni', [128, BLK])
            bia = cload('bia', [128, 10])
            vch_sb = cpool.tile([128, NCH], F32, tag="vch")
            nc.sync.dma_start(out=vch_sb[:], in_=vch.ap())
            ident = cpool.tile([128, 128], F32, tag="ident")
            make_identity(nc, ident[:])

            cb1, ncb1, ncb2a, cb2b = (bia[:, i:i + 1] for i in range(4))
            gb1, ngb1, gb2 = (bia[:, i:i + 1] for i in range(4, 7))
            fb1, nfb1, fb2 = (bia[:, i:i + 1] for i in range(7, 10))

            # group splits of a block's NCHB chunks into <=4-chunk matmul groups
            splits = []
            off = 0
            while off < NCHB:
                gw = min(4, NCHB - off)
                splits.append((off, gw))
                off += gw

            seg_tiles = [None] * NBLK

            def phase2_tile(t):
                vbase = t * 512
                vw = min(512, VSH - vbase)
                nblk_here = vw // BLK
                gx = p2pool.tile([GF, vw], F32R, tag="gx")
                nc.sync.dma_start(out=gx[:], in_=gt[:, vbase:vbase + vw].bitcast(F32R))
                gl1 = ps_l1.tile([128, vw], F32, tag="l1")
                nc.tensor.matmul(gl1[:], w_g1[:], gx[:], start=True, stop=True)
                gep = p2pool.tile([128, vw], F32, tag="gep")
                nc.scalar.activation(gep[:], gl1[:], AF.Exp, bias=ngb1, scale=-1.0)
                gden = p2pool.tile([128, vw], F32, tag="gden")
                nc.gpsimd.tensor_scalar(gden[:], gep[:], 1.0, None, ALU.add)
                grec = p2pool.tile([128, vw], F32, tag="grec")
                nc.vector.reciprocal(grec[:], gden[:])
                g1 = p2pool.tile([128, vw], F32R, tag="g1")
                nc.vector.scalar_tensor_tensor(g1[:], gl1[:], gb1, grec[:],
                                               ALU.add, ALU.mult)
                gl2 = ps_a.tile([128, vw], F32, tag="a")
                nc.tensor.matmul(gl2[:], w_g2[:], g1[:], start=True, stop=True)
                g2 = p2pool.tile([128, vw], F32R, tag="g2")
                nc.scalar.activation(g2[:], gl2[:], AF.Identity, bias=gb2, scale=1.0)

                fl1 = ps_b.tile([128, vw], F32, tag="b")
                nc.tensor.matmul(fl1[:], w_f1b[:], g2[:],
                                 start=True, stop=False, skip_group_check=True)
                for j in range(nblk_here):
                    nc.tensor.matmul(fl1[:, j * BLK:(j + 1) * BLK],
                                     w_f1a[:], seg_tiles[2 * t + j][:],
                                     start=False, stop=True, skip_group_check=True)
                fep = p2pool.tile([128, vw], F32, tag="fep")
                nc.scalar.activation(fep[:], fl1[:], AF.Exp, bias=nfb1, scale=-1.0)
                fden = p2pool.tile([128, vw], F32, tag="fden")
                nc.gpsimd.tensor_scalar(fden[:], fep[:], 1.0, None, ALU.add)
                frec = p2pool.tile([128, vw], F32, tag="frec")
                nc.vector.reciprocal(frec[:], fden[:])
                f1 = p2pool.tile([128, vw], F32R, tag="f1")
                nc.vector.scalar_tensor_tensor(f1[:], fl1[:], fb1, frec[:],
                                               ALU.add, ALU.mult)
                fl2 = ps_t.tile([128, vw], F32, tag="t")
                nc.tensor.matmul(fl2[:], w_f2[:], f1[:], start=True, stop=True)
                out = p2pool.tile([128, vw], F32, tag="out")
                nc.scalar.activation(out[:], fl2[:], AF.Identity, bias=fb2, scale=1.0)
                nc.sync.dma_start(out=hg[:, vbase:vbase + vw], in_=out[:])

            for b in range(NBLK):
                segps = ps_seg.tile([128, BLK], F32, tag="seg")
                ep = wpool.tile([128, R], F32, tag="ep")
                ea = wpool.tile([128, R], F32, tag="ea")
                eb = wpool.tile([128, R], F32, tag="eb")
                sp = wpool.tile([128, R], F32, tag="sp")
                rec1 = wpool.tile([128, R], F32, tag="rec1")
                rec2 = wpool.tile([128, R], F32, tag="rec2")
                h1 = wpool.tile([128, R], F32R, tag="h1")
                hch = wpool.tile([128, R], F32, tag="hch")
                l1g = {}
                for (goff, gw) in splits:
                    w = gw * 128
                    c0 = b * NCHB + goff
                    gs = slice(goff * 128, goff * 128 + w)
                    xtile = xpool.tile([CF, w], F32R, tag="x")
                    nc.sync.dma_start(out=xtile[:],
                                      in_=xt[:, c0 * 128:c0 * 128 + w].bitcast(F32R))
                    l1 = ps_l1.tile([128, w], F32, tag="l1")
                    nc.tensor.matmul(l1[:], w_c1[:], xtile[:], start=True, stop=True)
                    nc.scalar.activation(ep[:, gs], l1[:], AF.Exp, bias=ncb1, scale=-1.0)
                    l1g[goff] = l1
                nc.gpsimd.tensor_scalar(ep[:], ep[:], 1.0, None, ALU.add)
                nc.vector.reciprocal(rec1[:], ep[:])
                for (goff, gw) in splits:
                    w = gw * 128
                    gs = slice(goff * 128, goff * 128 + w)
                    nc.vector.scalar_tensor_tensor(h1[:, gs], l1g[goff][:], cb1,
                                                   rec1[:, gs], ALU.add, ALU.mult)
                    l2a = ps_a.tile([128, w], F32, tag="a")
                    nc.tensor.matmul(l2a[:], w_c2a[:], h1[:, gs], start=True, stop=True)
                    l2b = ps_b.tile([128, w], F32, tag="b")
                    nc.tensor.matmul(l2b[:], w_c2b[:], h1[:, gs], start=True, stop=True)
                    nc.scalar.activation(ea[:, gs], l2a[:], AF.Exp, bias=ncb2a, scale=-1.0)
                    nc.scalar.activation(eb[:, gs], l2b[:], AF.Exp, bias=cb2b, scale=1.0)
                nc.gpsimd.tensor_scalar(ea[:], ea[:], 1.0, None, ALU.add)
                nc.vector.reciprocal(rec2[:], ea[:])
                nc.scalar.activation(sp[:], eb[:], AF.Ln, bias=1.0, scale=1.0)
                nc.vector.tensor_tensor(out=hch[:], in0=sp[:], in1=rec2[:], op=ALU.mult)
                for (goff, gw) in splits:
                    w = gw * 128
                    c0 = b * NCHB + goff
                    tp = ps_t.tile([128, w], F32, tag="t")
                    for i in range(gw):
                        s = slice(i * 128, (i + 1) * 128)
                        bs = slice((goff + i) * 128, (goff + i + 1) * 128)
                        nc.tensor.transpose(tp[:, s], hch[:, bs], ident[:])
                    hrm = wpool.tile([128, w], F32R, tag="hrm")
                    nc.any.tensor_copy(out=hrm[:], in_=tp[:])
                    nc.sync.dma_start(
                        out=hc[c0 * 128:(c0 + gw) * 128, :].bitcast(F32R)
                            .rearrange("(g r) f -> r g f", g=gw),
                        in_=hrm[:].rearrange("p (g f) -> p g f", g=gw))
                    for i in range(gw):
                        c = c0 + i
                        oh = ohpool.tile([128, BLK], F32R, tag="oh")
                        nc.gpsimd.tensor_scalar(oh[:], ni_sb[:], vch_sb[:, c:c + 1],
                                                0.0, ALU.subtract, ALU.is_equal)
                        ci = goff + i
                        nc.tensor.matmul(segps[:], hrm[:, i * 128:(i + 1) * 128], oh[:],
                                         start=(ci == 0), stop=(ci == NCHB - 1))
                seg_sb = segsbpool.tile([128, BLK], F32R, tag="segsb")
                nc.any.tensor_copy(out=seg_sb[:], in_=segps[:])
                seg_tiles[b] = seg_sb
                if b % 2 == 1:
                    phase2_tile(b // 2)
            if NBLK % 2 == 1:
                phase2_tile(NBLK // 2)  # last 256-wide tile (block NBLK-1)

    nc.compile()
    return nc


def kernel(**inputs):
    global LAST_RESULTS
    i = {k: np.asarray(v) for k, v in inputs.items()}
    chem = np.ascontiguousarray(i['chem_feats'], dtype=np.float32)
    geom = np.ascontiguousarray(i['geom_feats'], dtype=np.float32)
    cores, R = _host_prepare(i['nbr_vids'], chem, geom)

    W = {}
    W['cW1'], W['cb1'] = _fold_bn(i['cW1'], i['cb1'], i['cg1'], i['cbe1'], i['cm1'], i['cv1'])
    cW2, cb2 = _fold_bn(i['cW2'], i['cb2'], i['cg2'], i['cbe2'], i['cm2'], i['cv2'])
    W['cW2a'], W['cb2a'] = np.ascontiguousarray(cW2[:, :H]), cb2[:H]
    W['cW2b'], W['cb2b'] = np.ascontiguousarray(cW2[:, H:]), cb2[H:]
    W['gW1'], W['gb1'] = _fold_bn(i['gW1'], i['gb1'], i['gg1'], i['gbe1'], i['gm1'], i['gv1'])
    W['gW2'], W['gb2'] = _fold_bn(i['gW2'], i['gb2'], i['gg2'], i['gbe2'], i['gm2'], i['gv2'])
    fW1, W['fb1'] = _fold_bn(i['fW1'], i['fb1'], i['fg1'], i['fbe1'], i['fm1'], i['fv1'])
    W['fW1a'], W['fW1b'] = np.ascontiguousarray(fW1[:H]), np.ascontiguousarray(fW1[H:])
    W['fW2'], W['fb2'] = _fold_bn(i['fW2'], i['fb2'], i['fg2'], i['fbe2'], i['fm2'], i['fv2'])

    nc = _build_program(R, W)

    global LAST_NC
    LAST_NC = nc
    in_maps = [{'xt': c['xT'], 'vch': c['vch'], 'gt': c['gT']} for c in cores]
    res = run_bass_kernel_spmd(nc, in_maps, core_ids=list(range(NCORES)), trace=TRACE)
    LAST_RESULTS = res

    h_chem = np.empty((N_ROWS, H), np.float32)
    h_geom = np.empty((V, H), np.float32)
    for k in range(NCORES):
        p = cores[k]['perm']
        real = p >= 0
        h_chem[p[real]] = res.results[k]['hc'][real]
        nreal = max(0, min(VSH, V - k * VSH))
        if nreal > 0:
            h_geom[k * VSH:k * VSH + nreal] = res.results[k]['hg'].T[:nreal]
    return h_geom, h_chem
